# revision 1
# baseline (speedup 1.0000x reference)
"""Trainium2 Bass kernel for a basic tanh RNN + output projection.

Reference computation (all fp32):
    s_t = tanh(x[:, :, t] @ Wx + s_{t-1} @ Wh + b)      t = 0..T-1, s_{-1} = 0
    out[:, t, :] = s_t @ Wout + bout

Shapes: x (64, 256, 1024), Wx (256, 1024), Wh (1024, 1024), b (1024,),
        Wout (1024, 512), bout (512,)  ->  out (64, 1024, 512)

Strategy (8 NeuronCores):
  The T=1024 recurrence is sequential; per step the PE must reload all 64
  Wh 128x128 stationary tiles, which costs the same whether a core carries
  8 or 64 batch rows.  So every core runs the full-batch recurrence
  (replicated; state kept transposed [H, B] on partitions so no per-step
  transposes are needed), and only the parallel work -- the output
  projection and the output writes -- is sharded by batch.  Each core
  receives x with the batch axis rotated so that its own 8 batch columns
  sit at positions 0..7; all cores then run one identical program (SPMD).

  Per step (all bf16 matmul inputs, fp32 PSUM accumulation): for each of 8
  hidden m-blocks, 2 Wx pairs + 8 Wh pairs of [128,128]x[128,64] matmuls
  accumulate z.T in PSUM, then ScalarE applies tanh(z+b) writing bf16
  state into parity-split windowed stage tiles (even/odd steps alternate
  tiles, avoiding false write-after-read hazards).  The Wh k-loop runs
  ascending so each group's early weight loads have stale (already
  satisfied) dependencies -- reordering this doubles runtime.  Every 64 steps
  the projection for the core's own 8 batch columns is folded in (moving
  dim 256 per parity half via a strided AP over the stage window); its
  bias-add/copy runs on VectorE -- keeping ScalarE exclusively on Tanh
  avoids ~1.3us activation-table reloads.  Measured 2.858 ms on hardware
  (vs ~2.2 ms PE-streaming floor), rel err 6.06e-3 vs the fp32 reference.
"""

import numpy as np
import ml_dtypes

import concourse.bass as bass
from concourse import bacc
import concourse.mybir as mybir
import concourse.tile as tile
from concourse.bass_utils import run_bass_kernel_spmd

B, F, T = 64, 256, 1024
H, O = 1024, 512
NCORES = 8
MB = B // NCORES  # own-batch columns per core (projection shard)
P = 128
KH, KF, MH, OBK = H // P, F // P, H // P, O // P  # 8, 2, 8, 4

BF16 = mybir.dt.bfloat16
F32 = mybir.dt.float32
np_bf16 = ml_dtypes.bfloat16


def build_program(
    t_steps: int = T,
    w_steps: int = 32,
    zbufs: int = 4,
    proj_every: int = 2,
    reps: int = 1,
    parity: bool = False,
    defer_k7: bool = False,
    sbufs: int = 2,
    g_dma: int = 1,
) -> bass.Bass:
    assert t_steps % w_steps == 0
    nw = t_steps // w_steps
    pw = w_steps * MB  # projection moving size per window

    nc = bacc.Bacc()

    xt_d = nc.declare_dram_parameter("xt", [t_steps, F, B], BF16, isOutput=False)
    wh_d = nc.declare_dram_parameter("wh", [H, H], BF16, isOutput=False)
    wx_d = nc.declare_dram_parameter("wx", [F, H], BF16, isOutput=False)
    wo_d = nc.declare_dram_parameter("wout", [H, O], BF16, isOutput=False)
    b_d = nc.declare_dram_parameter("bvec", [H], F32, isOutput=False)
    bo_d = nc.declare_dram_parameter("boutvec", [O], F32, isOutput=False)
    out_d = nc.declare_dram_parameter("out", [nw, OBK, P, pw], F32, isOutput=True)

    with tile.TileContext(nc) as tc:
        with (
            tc.tile_pool(name="const", bufs=1) as cpool,
            tc.tile_pool(name="stage", bufs=sbufs) as spool,
            tc.tile_pool(name="xin", bufs=max(2, 6 // g_dma)) as xpool,
            tc.tile_pool(name="outsb", bufs=4) as opool,
            tc.tile_pool(name="psz", bufs=zbufs, space="PSUM") as zpool,
            tc.tile_pool(name="psp", bufs=2, space="PSUM") as ppool,
        ):
            # --- resident weights ---------------------------------------
            wh_sb = cpool.tile([P, KH, H], BF16, tag="wh")
            nc.sync.dma_start(wh_sb[:], wh_d.rearrange("(kb p) c -> p kb c", p=P))
            wx_sb = cpool.tile([P, KF, H], BF16, tag="wx")
            nc.sync.dma_start(wx_sb[:], wx_d.rearrange("(kb p) c -> p kb c", p=P))
            wo_sb = cpool.tile([P, MH, O], BF16, tag="wo")
            nc.sync.dma_start(wo_sb[:], wo_d.rearrange("(kb p) c -> p kb c", p=P))
            b_sb = cpool.tile([P, KH], F32, tag="b")
            nc.sync.dma_start(b_sb[:], b_d.rearrange("(m p) -> p m", p=P))
            bo_sb = cpool.tile([P, OBK], F32, tag="bo")
            nc.sync.dma_start(bo_sb[:], bo_d.rearrange("(m p) -> p m", p=P))

            def emit_whole_kernel():
                stage_prev = None
                stage_cur = None
                pending_proj = []  # (window_idx, stage_tiles)

                def emit_proj_block(w_idx, stiles, ob):
                    pp = ppool.tile([P, pw], F32, tag="pproj", name="pproj")
                    if parity:
                        half = pw // 2
                        for par in range(2):
                            for m in range(MH):
                                nc.tensor.matmul(
                                    pp[:, par * half : (par + 1) * half],
                                    wo_sb[:, m, ob * P : (ob + 1) * P],
                                    stiles[m][par][:, :, 0:MB],
                                    start=(m == 0),
                                    stop=(m == MH - 1),
                                )
                    else:
                        for m in range(MH):
                            nc.tensor.matmul(
                                pp,
                                wo_sb[:, m, ob * P : (ob + 1) * P],
                                stiles[m][:, :, 0:MB],
                                start=(m == 0),
                                stop=(m == MH - 1),
                            )
                    osb = opool.tile([P, pw], F32, tag="osb", name="osb")
                    nc.vector.tensor_scalar_add(osb, pp, bo_sb[:, ob : ob + 1])
                    nc.sync.dma_start(out_d[w_idx, ob], osb)

                def state_slice(tiles_m, t_local):
                    if parity:
                        return tiles_m[t_local % 2][:, t_local // 2, :]
                    return tiles_m[:, t_local, :]

                for t in range(t_steps):
                    tl = t % w_steps
                    if tl == 0:
                        stage_prev = stage_cur
                        if parity:
                            stage_cur = [
                                [
                                    spool.tile(
                                        [P, w_steps // 2, B], BF16,
                                        tag=f"stage{m}p{par}", name=f"stage{m}p{par}",
                                    )
                                    for par in range(2)
                                ]
                                for m in range(MH)
                            ]
                        else:
                            stage_cur = [
                                spool.tile(
                                    [P, w_steps, B], BF16,
                                    tag=f"stage{m}", name=f"stage{m}",
                                )
                                for m in range(MH)
                            ]

                    if g_dma == 1:
                        xt_sb = xpool.tile([P, KF, B], BF16, tag="xt", name="xt")
                        nc.sync.dma_start(
                            xt_sb[:], xt_d[t].rearrange("(kb p) b -> p kb b", p=P)
                        )
                        xt_step = xt_sb
                    else:
                        if t % g_dma == 0:
                            xt_grp = xpool.tile(
                                [P, KF, g_dma, B], BF16, tag="xt", name="xt"
                            )
                            for kb in range(KF):
                                nc.sync.dma_start(
                                    xt_grp[:, kb],
                                    xt_d[
                                        bass.ds(t, g_dma),
                                        kb * P : (kb + 1) * P,
                                        :,
                                    ].rearrange("t p b -> p t b"),
                                )
                        xt_step = xt_grp[:, :, t % g_dma, :]

                    deferred = None  # (ps, prev, ptl) for group 0's k=7
                    for m in range(MH):
                        ps = zpool.tile([P, B], F32, tag="psz", name="psz")
                        do_defer = defer_k7 and m == 0 and t > 0
                        nlast = KF - 1 if t == 0 else KF + KH - 1
                        idx = 0
                        for kf in range(KF):
                            nc.tensor.matmul(
                                ps,
                                wx_sb[:, kf, m * P : (m + 1) * P],
                                xt_step[:, kf, :] if g_dma == 1 else xt_step[:, kf, :],
                                start=(idx == 0),
                                stop=(not do_defer and idx == nlast),
                            )
                            idx += 1
                        if t > 0:
                            prev = stage_cur if tl > 0 else stage_prev
                            ptl = (t - 1) % w_steps
                            ks = range(KH - 1) if do_defer else range(KH)
                            for k in ks:
                                nc.tensor.matmul(
                                    ps,
                                    wh_sb[:, k, m * P : (m + 1) * P],
                                    state_slice(prev[k], ptl),
                                    start=False,
                                    stop=(not do_defer and idx == nlast),
                                )
                                idx += 1
                        if do_defer:
                            deferred = (ps, prev, ptl)
                            continue
                        nc.scalar.activation(
                            state_slice(stage_cur[m], tl),
                            ps,
                            mybir.ActivationFunctionType.Tanh,
                            bias=b_sb[:, m : m + 1],
                        )
                        if deferred is not None and m == 1:
                            ps0, prev0, ptl0 = deferred
                            deferred = None
                            nc.tensor.matmul(
                                ps0,
                                wh_sb[:, KH - 1, 0:P],
                                state_slice(prev0[KH - 1], ptl0),
                                start=False,
                                stop=True,
                            )
                            nc.scalar.activation(
                                state_slice(stage_cur[0], tl),
                                ps0,
                                mybir.ActivationFunctionType.Tanh,
                                bias=b_sb[:, 0:1],
                            )

                    if (
                        pending_proj
                        and tl % proj_every == proj_every - 1
                        and tl // proj_every < OBK
                    ):
                        emit_proj_block(
                            pending_proj[0][0], pending_proj[0][1], tl // proj_every
                        )
                        if tl // proj_every == OBK - 1:
                            pending_proj.pop(0)

                    if tl == w_steps - 1:
                        pending_proj.append((t // w_steps, stage_cur))

                for w_idx, stiles in pending_proj:
                    for ob in range(OBK):
                        emit_proj_block(w_idx, stiles, ob)

            if reps > 1:
                with tc.For_i(0, reps, 1):
                    emit_whole_kernel()
            else:
                emit_whole_kernel()

    nc.compile()
    return nc


def _host_prep(x, Wx, Wh, b, Wout, bout, t_steps):
    """Build the 8 per-core input maps."""
    xt = np.ascontiguousarray(x[:, :, :t_steps].transpose(2, 1, 0)).astype(np_bf16)
    wh = Wh.astype(np_bf16)
    wx = Wx.astype(np_bf16)
    wo = Wout.astype(np_bf16)
    bv = np.ascontiguousarray(b, dtype=np.float32)
    bo = np.ascontiguousarray(bout, dtype=np.float32)
    in_maps = []
    for c in range(NCORES):
        xt_c = np.ascontiguousarray(np.roll(xt, -MB * c, axis=2))
        in_maps.append(
            {
                "xt": xt_c,
                "wh": wh,
                "wx": wx,
                "wout": wo,
                "bvec": bv,
                "boutvec": bo,
            }
        )
    return in_maps


def _assemble(results, t_steps, w_steps, parity=False):
    nw = t_steps // w_steps
    out = np.empty((B, t_steps, O), np.float32)
    for c in range(NCORES):
        if parity:
            arr = results[c]["out"].reshape(nw, OBK, P, 2, w_steps // 2, MB)
            # out[MB*c+j, w*W + tt*2 + par, ob*P + p] = arr[w, ob, p, par, tt, j]
            out[MB * c : MB * (c + 1)] = (
                arr.transpose(5, 0, 4, 3, 1, 2).reshape(MB, t_steps, O)
            )
        else:
            arr = results[c]["out"].reshape(nw, OBK, P, w_steps, MB)
            out[MB * c : MB * (c + 1)] = (
                arr.transpose(4, 0, 3, 1, 2).reshape(MB, t_steps, O)
            )
    return out


def run(
    x, Wx, Wh, b, Wout, bout,
    t_steps=T, w_steps=64, zbufs=4, parity=True, trace=False,
):
    nc = build_program(t_steps, w_steps, zbufs=zbufs, parity=parity)
    in_maps = _host_prep(x, Wx, Wh, b, Wout, bout, t_steps)
    res = run_bass_kernel_spmd(nc, in_maps, list(range(NCORES)), trace=trace)
    out = _assemble(res.results, t_steps, w_steps, parity=parity)
    return out, res


def kernel(x, Wx, Wh, b, Wout, bout):
    out, _ = run(
        np.asarray(x, dtype=np.float32),
        np.asarray(Wx, dtype=np.float32),
        np.asarray(Wh, dtype=np.float32),
        np.asarray(b, dtype=np.float32),
        np.asarray(Wout, dtype=np.float32),
        np.asarray(bout, dtype=np.float32),
    )
    return out



# revision 17
# speedup vs baseline: 1.3659x; 1.3659x over previous
"""Trainium2 Bass kernel for a basic tanh RNN + output projection.

Reference computation (all fp32):
    s_t = tanh(x[:, :, t] @ Wx + s_{t-1} @ Wh + b)      t = 0..T-1, s_{-1} = 0
    out[:, t, :] = s_t @ Wout + bout

Shapes: x (64, 256, 1024), Wx (256, 1024), Wh (1024, 1024), b (1024,),
        Wout (1024, 512), bout (512,)  ->  out (64, 1024, 512)

Strategy (8 NeuronCores):
  The T=1024 recurrence is sequential; per step the PE must reload all 80
  [128,128] stationary tiles (64 Wh + 16 Wx), which costs the same whether
  a core carries 8 or 64 batch rows, and per-step cross-core exchange is
  impossible (collectives have a ~20us latency floor).  So every core runs
  the full-batch recurrence (replicated; state kept transposed [H, B] on
  partitions so no per-step transposes are needed), and only the parallel
  work -- the output projection and the output writes -- is sharded by
  batch.  Each core receives x with the batch axis rotated so that its own
  8 batch columns sit at positions 0..7; all cores run one identical
  program (SPMD).

  Per step (bf16 matmul inputs, fp32 PSUM): for each of 8 hidden m-blocks,
  2 Wx + 8 Wh matmuls of [128,128]x[128,64] accumulate z.T in PSUM.  The
  previous version then ran one ScalarE tanh per m-block; at [128,64] an
  ACTIVATE costs (64+352)/1.2 ns -- the 352-cycle fixed overhead dominates
  -- so 8 per step saturate ScalarE at 2.77us/step, which matched the
  measured 2.86 ms almost exactly (ScalarE-bound, not PE-bound).  Now the
  z accumulators of several m-blocks share a PSUM bank (act groups
  m0 | m1..m5 | m6 | m7) and ONE ACTIVATE covers each group, cutting
  ScalarE to ~1.6us/step.  The trailing groups stay singletons so the tanh
  feeding the next step's k=6,7 matmuls is short (347 ns), and m0's k=7
  matmul is deferred until after m1's group, giving tanh(m7) time to land
  (m0 must then own a bank by itself: start=True clears has_written for
  the whole bank, which would corrupt a deferred accumulate into a shared
  bank).  Group m1..m5's ACT crosses the step boundary, so its bank (and
  m6's) is double-buffered against the next step's PE writes; singleton
  banks are reused every step (their ACTs retire early).  Stage state is
  one [128, w/2, 8, 64] bf16 tile per parity (even/odd steps alternate
  tiles, avoiding false WAR hazards); an ACT group writes [128, g*64]
  contiguously and the projection reads [w/2, 8] strided APs per m.  The
  Wh k-loop runs ascending so early weight loads have already-satisfied
  dependencies.  Every 64 steps the projection for the core's own 8 batch
  columns is folded in; its bias-add runs on VectorE -- keeping ScalarE
  exclusively on Tanh avoids ~2.7us activation-table reloads.  Batched
  tanh cannot apply a per-m bias ([P,1] only), so this path requires
  b == 0 (true per the spec, fill=zeros); kernel() falls back to the
  per-m-tanh program for nonzero b.  Local slope-measured 2.87 ms (from
  2.86-3.4 ms baseline depending on device state); on ScalarE-bound
  hardware the model predicts ~2.3-2.5 ms.  rel err 6.05e-3 (bit-identical
  math to the previous version).
"""

import numpy as np
import ml_dtypes

import concourse.bass as bass
from concourse import bacc
import concourse.mybir as mybir
import concourse.tile as tile
from concourse.bass_utils import run_bass_kernel_spmd

B, F, T = 64, 256, 1024
H, O = 1024, 512
NCORES = 8
MB = B // NCORES  # own-batch columns per core (projection shard)
P = 128
KH, KF, MH, OBK = H // P, F // P, H // P, O // P  # 8, 2, 8, 4

BF16 = mybir.dt.bfloat16
F32 = mybir.dt.float32
np_bf16 = ml_dtypes.bfloat16


def build_program(
    t_steps: int = T,
    w_steps: int = 32,
    zbufs: int = 4,
    proj_every: int = 2,
    reps: int = 1,
    parity: bool = False,
    defer_k7: bool = False,
    sbufs: int = 2,
    g_dma: int = 1,
) -> bass.Bass:
    assert t_steps % w_steps == 0
    nw = t_steps // w_steps
    pw = w_steps * MB  # projection moving size per window

    nc = bacc.Bacc()

    xt_d = nc.declare_dram_parameter("xt", [t_steps, F, B], BF16, isOutput=False)
    wh_d = nc.declare_dram_parameter("wh", [H, H], BF16, isOutput=False)
    wx_d = nc.declare_dram_parameter("wx", [F, H], BF16, isOutput=False)
    wo_d = nc.declare_dram_parameter("wout", [H, O], BF16, isOutput=False)
    b_d = nc.declare_dram_parameter("bvec", [H], F32, isOutput=False)
    bo_d = nc.declare_dram_parameter("boutvec", [O], F32, isOutput=False)
    out_d = nc.declare_dram_parameter("out", [nw, OBK, P, pw], F32, isOutput=True)

    with tile.TileContext(nc) as tc:
        with (
            tc.tile_pool(name="const", bufs=1) as cpool,
            tc.tile_pool(name="stage", bufs=sbufs) as spool,
            tc.tile_pool(name="xin", bufs=max(2, 6 // g_dma)) as xpool,
            tc.tile_pool(name="outsb", bufs=4) as opool,
            tc.tile_pool(name="psz", bufs=zbufs, space="PSUM") as zpool,
            tc.tile_pool(name="psp", bufs=2, space="PSUM") as ppool,
        ):
            # --- resident weights ---------------------------------------
            wh_sb = cpool.tile([P, KH, H], BF16, tag="wh")
            nc.sync.dma_start(wh_sb[:], wh_d.rearrange("(kb p) c -> p kb c", p=P))
            wx_sb = cpool.tile([P, KF, H], BF16, tag="wx")
            nc.sync.dma_start(wx_sb[:], wx_d.rearrange("(kb p) c -> p kb c", p=P))
            wo_sb = cpool.tile([P, MH, O], BF16, tag="wo")
            nc.sync.dma_start(wo_sb[:], wo_d.rearrange("(kb p) c -> p kb c", p=P))
            b_sb = cpool.tile([P, KH], F32, tag="b")
            nc.sync.dma_start(b_sb[:], b_d.rearrange("(m p) -> p m", p=P))
            bo_sb = cpool.tile([P, OBK], F32, tag="bo")
            nc.sync.dma_start(bo_sb[:], bo_d.rearrange("(m p) -> p m", p=P))

            def emit_whole_kernel():
                stage_prev = None
                stage_cur = None
                pending_proj = []  # (window_idx, stage_tiles)

                def emit_proj_block(w_idx, stiles, ob):
                    pp = ppool.tile([P, pw], F32, tag="pproj", name="pproj")
                    if parity:
                        half = pw // 2
                        for par in range(2):
                            for m in range(MH):
                                nc.tensor.matmul(
                                    pp[:, par * half : (par + 1) * half],
                                    wo_sb[:, m, ob * P : (ob + 1) * P],
                                    stiles[m][par][:, :, 0:MB],
                                    start=(m == 0),
                                    stop=(m == MH - 1),
                                )
                    else:
                        for m in range(MH):
                            nc.tensor.matmul(
                                pp,
                                wo_sb[:, m, ob * P : (ob + 1) * P],
                                stiles[m][:, :, 0:MB],
                                start=(m == 0),
                                stop=(m == MH - 1),
                            )
                    osb = opool.tile([P, pw], F32, tag="osb", name="osb")
                    nc.vector.tensor_scalar_add(osb, pp, bo_sb[:, ob : ob + 1])
                    nc.sync.dma_start(out_d[w_idx, ob], osb)

                def state_slice(tiles_m, t_local):
                    if parity:
                        return tiles_m[t_local % 2][:, t_local // 2, :]
                    return tiles_m[:, t_local, :]

                for t in range(t_steps):
                    tl = t % w_steps
                    if tl == 0:
                        stage_prev = stage_cur
                        if parity:
                            stage_cur = [
                                [
                                    spool.tile(
                                        [P, w_steps // 2, B], BF16,
                                        tag=f"stage{m}p{par}", name=f"stage{m}p{par}",
                                    )
                                    for par in range(2)
                                ]
                                for m in range(MH)
                            ]
                        else:
                            stage_cur = [
                                spool.tile(
                                    [P, w_steps, B], BF16,
                                    tag=f"stage{m}", name=f"stage{m}",
                                )
                                for m in range(MH)
                            ]

                    if g_dma == 1:
                        xt_sb = xpool.tile([P, KF, B], BF16, tag="xt", name="xt")
                        nc.sync.dma_start(
                            xt_sb[:], xt_d[t].rearrange("(kb p) b -> p kb b", p=P)
                        )
                        xt_step = xt_sb
                    else:
                        if t % g_dma == 0:
                            xt_grp = xpool.tile(
                                [P, KF, g_dma, B], BF16, tag="xt", name="xt"
                            )
                            for kb in range(KF):
                                nc.sync.dma_start(
                                    xt_grp[:, kb],
                                    xt_d[
                                        bass.ds(t, g_dma),
                                        kb * P : (kb + 1) * P,
                                        :,
                                    ].rearrange("t p b -> p t b"),
                                )
                        xt_step = xt_grp[:, :, t % g_dma, :]

                    deferred = None  # (ps, prev, ptl) for group 0's k=7
                    for m in range(MH):
                        ps = zpool.tile([P, B], F32, tag="psz", name="psz")
                        do_defer = defer_k7 and m == 0 and t > 0
                        nlast = KF - 1 if t == 0 else KF + KH - 1
                        idx = 0
                        for kf in range(KF):
                            nc.tensor.matmul(
                                ps,
                                wx_sb[:, kf, m * P : (m + 1) * P],
                                xt_step[:, kf, :] if g_dma == 1 else xt_step[:, kf, :],
                                start=(idx == 0),
                                stop=(not do_defer and idx == nlast),
                            )
                            idx += 1
                        if t > 0:
                            prev = stage_cur if tl > 0 else stage_prev
                            ptl = (t - 1) % w_steps
                            ks = range(KH - 1) if do_defer else range(KH)
                            for k in ks:
                                nc.tensor.matmul(
                                    ps,
                                    wh_sb[:, k, m * P : (m + 1) * P],
                                    state_slice(prev[k], ptl),
                                    start=False,
                                    stop=(not do_defer and idx == nlast),
                                )
                                idx += 1
                        if do_defer:
                            deferred = (ps, prev, ptl)
                            continue
                        nc.scalar.activation(
                            state_slice(stage_cur[m], tl),
                            ps,
                            mybir.ActivationFunctionType.Tanh,
                            bias=b_sb[:, m : m + 1],
                        )
                        if deferred is not None and m == 1:
                            ps0, prev0, ptl0 = deferred
                            deferred = None
                            nc.tensor.matmul(
                                ps0,
                                wh_sb[:, KH - 1, 0:P],
                                state_slice(prev0[KH - 1], ptl0),
                                start=False,
                                stop=True,
                            )
                            nc.scalar.activation(
                                state_slice(stage_cur[0], tl),
                                ps0,
                                mybir.ActivationFunctionType.Tanh,
                                bias=b_sb[:, 0:1],
                            )

                    if (
                        pending_proj
                        and tl % proj_every == proj_every - 1
                        and tl // proj_every < OBK
                    ):
                        emit_proj_block(
                            pending_proj[0][0], pending_proj[0][1], tl // proj_every
                        )
                        if tl // proj_every == OBK - 1:
                            pending_proj.pop(0)

                    if tl == w_steps - 1:
                        pending_proj.append((t // w_steps, stage_cur))

                for w_idx, stiles in pending_proj:
                    for ob in range(OBK):
                        emit_proj_block(w_idx, stiles, ob)

            if reps > 1:
                with tc.For_i(0, reps, 1):
                    emit_whole_kernel()
            else:
                emit_whole_kernel()

    nc.compile()
    return nc


def build_program2(
    t_steps: int = T,
    w_steps: int = 64,
    zbufs: int = 1,
    proj_every: int = 2,
    reps: int = 1,
    defer_k7: bool = True,
    sbufs: int = 2,
    act_split: tuple = (1, 5, 1, 1),
    zgroup_bufs: tuple | None = None,
    g_dma: int = 1,
) -> bass.Bass:
    """Batched-activation variant.

    The baseline is ScalarE-bound: 8 per-step ACTIVATEs of [128,64] cost
    8*(64+352)/1.2 = 2.77us/step (the 352-cycle fixed overhead dominates at
    N=64), which matches the measured 2.86ms almost exactly.  Here the
    per-step z accumulators for several m-blocks share one PSUM bank
    ([128, g*64] f32 <= 2KB) so one ACTIVATE covers g m-blocks.  act_split
    partitions the 8 m-blocks into consecutive ACT groups; late groups stay
    small so the tanh feeding the next step's high-k matmuls is short.
    Requires b == 0 (no per-m bias possible in a batched ACT) -- the caller
    falls back to build_program otherwise.  Stage tiles merge all m-blocks
    ([P, w/2, KH, B] per parity) so one ACT write covers a group.
    """
    assert t_steps % w_steps == 0
    assert sum(act_split) == MH
    # start=True clears has_written for the WHOLE bank, so the deferred k7
    # (start=False, emitted after m1's group) only accumulates correctly if
    # m0 has its own bank -- i.e. act group 0 must be the singleton {m0}.
    assert not defer_k7 or act_split[0] == 1, "defer_k7 needs act_split[0]==1"
    if zgroup_bufs is None:
        # singleton groups' ACTs complete with big slack before their bank is
        # rewritten next step; only big groups (ACT near the step boundary)
        # need double-buffering against the next step's PE writes.
        zgroup_bufs = tuple(zbufs if g == 1 else max(zbufs, 2) for g in act_split)
    assert len(zgroup_bufs) == len(act_split)
    nw = t_steps // w_steps
    wh_half = w_steps // 2
    pw = w_steps * MB  # projection moving size per window
    half = pw // 2
    # m-group boundaries
    bounds = []
    lo = 0
    for g in act_split:
        bounds.append((lo, lo + g))
        lo += g
    grp_of = {}
    for gi, (mlo, mhi) in enumerate(bounds):
        for m in range(mlo, mhi):
            grp_of[m] = gi

    nc = bacc.Bacc()

    xt_d = nc.declare_dram_parameter("xt", [t_steps, F, B], BF16, isOutput=False)
    wh_d = nc.declare_dram_parameter("wh", [H, H], BF16, isOutput=False)
    wx_d = nc.declare_dram_parameter("wx", [F, H], BF16, isOutput=False)
    wo_d = nc.declare_dram_parameter("wout", [H, O], BF16, isOutput=False)
    bo_d = nc.declare_dram_parameter("boutvec", [O], F32, isOutput=False)
    out_d = nc.declare_dram_parameter("out", [nw, OBK, P, pw], F32, isOutput=True)

    with tile.TileContext(nc) as tc:
        with (
            tc.tile_pool(name="const", bufs=1) as cpool,
            tc.tile_pool(name="stage", bufs=sbufs) as spool,
            tc.tile_pool(name="xin", bufs=6) as xpool,
            tc.tile_pool(name="outsb", bufs=4) as opool,
            tc.tile_pool(name="psz1", bufs=1, space="PSUM") as zpool1,
            tc.tile_pool(name="psz2", bufs=2, space="PSUM") as zpool2,
            tc.tile_pool(name="psz3", bufs=3, space="PSUM") as zpool3,
            tc.tile_pool(name="psp", bufs=2, space="PSUM") as ppool,
        ):
            zpools = {1: zpool1, 2: zpool2, 3: zpool3}
            wh_sb = cpool.tile([P, KH, H], BF16, tag="wh")
            nc.sync.dma_start(wh_sb[:], wh_d.rearrange("(kb p) c -> p kb c", p=P))
            wx_sb = cpool.tile([P, KF, H], BF16, tag="wx")
            nc.sync.dma_start(wx_sb[:], wx_d.rearrange("(kb p) c -> p kb c", p=P))
            wo_sb = cpool.tile([P, MH, O], BF16, tag="wo")
            nc.sync.dma_start(wo_sb[:], wo_d.rearrange("(kb p) c -> p kb c", p=P))
            bo_sb = cpool.tile([P, OBK], F32, tag="bo")
            nc.sync.dma_start(bo_sb[:], bo_d.rearrange("(m p) -> p m", p=P))

            def emit_whole_kernel():
                stage_prev = None
                stage_cur = None
                pending_proj = []

                def emit_proj_block(w_idx, stiles, ob):
                    pp = ppool.tile([P, pw], F32, tag="pproj", name="pproj")
                    for par in range(2):
                        for m in range(MH):
                            nc.tensor.matmul(
                                pp[:, par * half : (par + 1) * half],
                                wo_sb[:, m, ob * P : (ob + 1) * P],
                                stiles[par][:, :, m, 0:MB],
                                start=(m == 0),
                                stop=(m == MH - 1),
                            )
                    osb = opool.tile([P, pw], F32, tag="osb", name="osb")
                    nc.vector.tensor_scalar_add(osb, pp, bo_sb[:, ob : ob + 1])
                    nc.sync.dma_start(out_d[w_idx, ob], osb)

                for t in range(t_steps):
                    tl = t % w_steps
                    par, row = tl % 2, tl // 2
                    if tl == 0:
                        stage_prev = stage_cur
                        stage_cur = [
                            spool.tile(
                                [P, wh_half, KH, B], BF16,
                                tag=f"stage{pp_}", name=f"stage{pp_}",
                            )
                            for pp_ in range(2)
                        ]

                    if g_dma == 1:
                        xt_sb = xpool.tile([P, KF, B], BF16, tag="xt", name="xt")
                        nc.sync.dma_start(
                            xt_sb[:], xt_d[t].rearrange("(kb p) b -> p kb b", p=P)
                        )
                        xt_step = xt_sb
                    else:
                        if t % g_dma == 0:
                            xt_grp = xpool.tile(
                                [P, KF, g_dma, B], BF16, tag="xt", name="xt"
                            )
                            for kb in range(KF):
                                nc.sync.dma_start(
                                    xt_grp[:, kb],
                                    xt_d[
                                        bass.ds(t, g_dma),
                                        kb * P : (kb + 1) * P,
                                        :,
                                    ].rearrange("t p b -> p t b"),
                                )
                        xt_step = xt_grp[:, :, t % g_dma, :]

                    if t > 0:
                        if tl > 0:
                            ppar, prow = (tl - 1) % 2, (tl - 1) // 2
                            prev_t = stage_cur
                        else:
                            ppar, prow = (w_steps - 1) % 2, (w_steps - 1) // 2
                            prev_t = stage_prev

                    ztiles = [
                        zpools[zgroup_bufs[gi]].tile(
                            [P, (mhi - mlo) * B], F32, tag=f"z{gi}",
                            name=f"z{gi}",
                        )
                        for gi, (mlo, mhi) in enumerate(bounds)
                    ]

                    deferred = None  # (zslice,) for m=0's k=KH-1
                    for m in range(MH):
                        gi = grp_of[m]
                        mlo, mhi = bounds[gi]
                        zs = ztiles[gi][:, (m - mlo) * B : (m - mlo + 1) * B]
                        do_defer = defer_k7 and m == 0 and t > 0
                        nlast = KF - 1 if t == 0 else KF + KH - 1
                        idx = 0
                        for kf in range(KF):
                            nc.tensor.matmul(
                                zs,
                                wx_sb[:, kf, m * P : (m + 1) * P],
                                xt_step[:, kf, :],
                                start=(idx == 0),
                                stop=(not do_defer and idx == nlast),
                            )
                            idx += 1
                        if t > 0:
                            ks = range(KH - 1) if do_defer else range(KH)
                            for k in ks:
                                nc.tensor.matmul(
                                    zs,
                                    wh_sb[:, k, m * P : (m + 1) * P],
                                    prev_t[ppar][:, prow, k, :],
                                    start=False,
                                    stop=(not do_defer and idx == nlast),
                                )
                                idx += 1
                        if do_defer:
                            deferred = zs
                        if deferred is not None and m == 1:
                            nc.tensor.matmul(
                                deferred,
                                wh_sb[:, KH - 1, 0:P],
                                prev_t[ppar][:, prow, KH - 1, :],
                                start=False,
                                stop=True,
                            )
                            deferred = None
                        # emit the ACT for any group whose last m-block is done;
                        # a singleton group 0 with a deferred k7 completes at m=1
                        for gj, (glo, ghi) in enumerate(bounds):
                            ready_at = ghi - 1
                            if defer_k7 and t > 0 and gj == 0:
                                ready_at = max(ready_at, 1)
                            if ready_at == m:
                                nc.scalar.activation(
                                    stage_cur[par][:, row, glo:ghi, :],
                                    ztiles[gj][:, 0 : (ghi - glo) * B],
                                    mybir.ActivationFunctionType.Tanh,
                                )

                    if (
                        pending_proj
                        and tl % proj_every == proj_every - 1
                        and tl // proj_every < OBK
                    ):
                        emit_proj_block(
                            pending_proj[0][0], pending_proj[0][1], tl // proj_every
                        )
                        if tl // proj_every == OBK - 1:
                            pending_proj.pop(0)

                    if tl == w_steps - 1:
                        pending_proj.append((t // w_steps, stage_cur))

                for w_idx, stiles in pending_proj:
                    for ob in range(OBK):
                        emit_proj_block(w_idx, stiles, ob)

            if reps > 1:
                with tc.For_i(0, reps, 1):
                    emit_whole_kernel()
            else:
                emit_whole_kernel()

    nc.compile()
    return nc


def _host_prep(x, Wx, Wh, b, Wout, bout, t_steps):
    """Build the 8 per-core input maps."""
    xt = np.ascontiguousarray(x[:, :, :t_steps].transpose(2, 1, 0)).astype(np_bf16)
    wh = Wh.astype(np_bf16)
    wx = Wx.astype(np_bf16)
    wo = Wout.astype(np_bf16)
    bv = np.ascontiguousarray(b, dtype=np.float32)
    bo = np.ascontiguousarray(bout, dtype=np.float32)
    in_maps = []
    for c in range(NCORES):
        xt_c = np.ascontiguousarray(np.roll(xt, -MB * c, axis=2))
        in_maps.append(
            {
                "xt": xt_c,
                "wh": wh,
                "wx": wx,
                "wout": wo,
                "bvec": bv,
                "boutvec": bo,
            }
        )
    return in_maps


def _assemble(results, t_steps, w_steps, parity=False):
    nw = t_steps // w_steps
    out = np.empty((B, t_steps, O), np.float32)
    for c in range(NCORES):
        if parity:
            arr = results[c]["out"].reshape(nw, OBK, P, 2, w_steps // 2, MB)
            # out[MB*c+j, w*W + tt*2 + par, ob*P + p] = arr[w, ob, p, par, tt, j]
            out[MB * c : MB * (c + 1)] = (
                arr.transpose(5, 0, 4, 3, 1, 2).reshape(MB, t_steps, O)
            )
        else:
            arr = results[c]["out"].reshape(nw, OBK, P, w_steps, MB)
            out[MB * c : MB * (c + 1)] = (
                arr.transpose(4, 0, 3, 1, 2).reshape(MB, t_steps, O)
            )
    return out


def build_final(reps: int = 1, v2: bool = True) -> bass.Bass:
    """The shipped program (v2 defaults); old path for nonzero b."""
    if v2:
        return build_program2(T, 64, reps=reps, zgroup_bufs=(1, 2, 2, 1))
    return build_program(T, 64, zbufs=6, parity=True, reps=reps)


def run(
    x, Wx, Wh, b, Wout, bout,
    t_steps=T, w_steps=64, zbufs=4, parity=True, trace=False, v2=None,
    v2_kwargs=None,
):
    if v2 is None:
        v2 = not np.any(np.asarray(b))
    if v2:
        kw = {"zgroup_bufs": (1, 2, 2, 1)}
        kw.update(v2_kwargs or {})
        nc = build_program2(t_steps, w_steps, **kw)
    else:
        nc = build_program(t_steps, w_steps, zbufs=zbufs, parity=parity)
    in_maps = _host_prep(x, Wx, Wh, b, Wout, bout, t_steps)
    if v2:
        for m in in_maps:
            m.pop("bvec", None)
    res = run_bass_kernel_spmd(nc, in_maps, list(range(NCORES)), trace=trace)
    out = _assemble(res.results, t_steps, w_steps, parity=True)
    return out, res


def kernel(x, Wx, Wh, b, Wout, bout):
    out, _ = run(
        np.asarray(x, dtype=np.float32),
        np.asarray(Wx, dtype=np.float32),
        np.asarray(Wh, dtype=np.float32),
        np.asarray(b, dtype=np.float32),
        np.asarray(Wout, dtype=np.float32),
        np.asarray(bout, dtype=np.float32),
    )
    return out



# revision 18
# speedup vs baseline: 1.6380x; 1.1993x over previous
"""Trainium2 Bass kernel for a basic tanh RNN + output projection.

Reference computation (all fp32):
    s_t = tanh(x[:, :, t] @ Wx + s_{t-1} @ Wh + b)      t = 0..T-1, s_{-1} = 0
    out[:, t, :] = s_t @ Wout + bout

Shapes: x (64, 256, 1024), Wx (256, 1024), Wh (1024, 1024), b (1024,),
        Wout (1024, 512), bout (512,)  ->  out (64, 1024, 512)

Strategy (8 NeuronCores):
  The T=1024 recurrence is sequential; per step the PE must reload all 80
  [128,128] stationary tiles (64 Wh + 16 Wx), which costs the same whether
  a core carries 8 or 64 batch rows, and per-step cross-core exchange is
  impossible (collectives have a ~20us latency floor).  So every core runs
  the full-batch recurrence (replicated; state kept transposed [H, B] on
  partitions so no per-step transposes are needed), and only the parallel
  work -- the output projection and the output writes -- is sharded by
  batch.  Each core receives x with the batch axis rotated so that its own
  8 batch columns sit at positions 0..7; all cores run one identical
  program (SPMD).

  Per step (bf16 matmul inputs, fp32 PSUM): for each of 8 hidden m-blocks,
  2 Wx + 8 Wh matmuls of [128,128]x[128,64] accumulate z.T in PSUM.  The
  previous version then ran one ScalarE tanh per m-block; at [128,64] an
  ACTIVATE costs (64+352)/1.2 ns -- the 352-cycle fixed overhead dominates
  -- so 8 per step saturate ScalarE at 2.77us/step, which matched the
  measured 2.86 ms almost exactly (ScalarE-bound, not PE-bound).  Now the
  z accumulators of several m-blocks share a PSUM bank (act groups
  m0 | m1..m5 | m6 | m7) and ONE ACTIVATE covers each group, cutting
  ScalarE to ~1.6us/step.  The trailing groups stay singletons so the tanh
  feeding the next step's k=6,7 matmuls is short (347 ns), and m0's k=7
  matmul is deferred until after m1's group, giving tanh(m7) time to land
  (m0 must then own a bank by itself: start=True clears has_written for
  the whole bank, which would corrupt a deferred accumulate into a shared
  bank).  Group m1..m5's ACT crosses the step boundary, so its bank (and
  m6's) is double-buffered against the next step's PE writes; singleton
  banks are reused every step (their ACTs retire early).  Stage state is
  one [128, w/2, 8, 64] bf16 tile per parity (even/odd steps alternate
  tiles, avoiding false WAR hazards); an ACT group writes [128, g*64]
  contiguously and the projection reads [w/2, 8] strided APs per m.  The
  Wh k-loop runs ascending so early weight loads have already-satisfied
  dependencies.  Every 64 steps the projection for the core's own 8 batch
  columns is folded in; its bias-add runs on VectorE -- keeping ScalarE
  exclusively on Tanh avoids ~2.7us activation-table reloads.  Batched
  tanh cannot apply a per-m bias ([P,1] only), so this path requires
  b == 0 (true per the spec, fill=zeros); kernel() falls back to the
  per-m-tanh program for nonzero b.  Local slope-measured 2.87 ms clean /
  3.45 ms in a slow device phase (the baseline measures 2.86-4.7 ms under
  the same drift); on ScalarE-bound hardware the model predicts
  ~2.3-2.5 ms.  rel err 6.05e-3 (bit-identical math to the previous
  version).
"""

import numpy as np
import ml_dtypes

import concourse.bass as bass
from concourse import bacc
import concourse.mybir as mybir
import concourse.tile as tile
from concourse.bass_utils import run_bass_kernel_spmd

B, F, T = 64, 256, 1024
H, O = 1024, 512
NCORES = 8
MB = B // NCORES  # own-batch columns per core (projection shard)
P = 128
KH, KF, MH, OBK = H // P, F // P, H // P, O // P  # 8, 2, 8, 4

BF16 = mybir.dt.bfloat16
F32 = mybir.dt.float32
np_bf16 = ml_dtypes.bfloat16


def build_program(
    t_steps: int = T,
    w_steps: int = 32,
    zbufs: int = 4,
    proj_every: int = 2,
    reps: int = 1,
    parity: bool = False,
    defer_k7: bool = False,
    sbufs: int = 2,
    g_dma: int = 1,
) -> bass.Bass:
    assert t_steps % w_steps == 0
    nw = t_steps // w_steps
    pw = w_steps * MB  # projection moving size per window

    nc = bacc.Bacc()

    xt_d = nc.declare_dram_parameter("xt", [t_steps, F, B], BF16, isOutput=False)
    wh_d = nc.declare_dram_parameter("wh", [H, H], BF16, isOutput=False)
    wx_d = nc.declare_dram_parameter("wx", [F, H], BF16, isOutput=False)
    wo_d = nc.declare_dram_parameter("wout", [H, O], BF16, isOutput=False)
    b_d = nc.declare_dram_parameter("bvec", [H], F32, isOutput=False)
    bo_d = nc.declare_dram_parameter("boutvec", [O], F32, isOutput=False)
    out_d = nc.declare_dram_parameter("out", [nw, OBK, P, pw], F32, isOutput=True)

    with tile.TileContext(nc) as tc:
        with (
            tc.tile_pool(name="const", bufs=1) as cpool,
            tc.tile_pool(name="stage", bufs=sbufs) as spool,
            tc.tile_pool(name="xin", bufs=max(2, 6 // g_dma)) as xpool,
            tc.tile_pool(name="outsb", bufs=4) as opool,
            tc.tile_pool(name="psz", bufs=zbufs, space="PSUM") as zpool,
            tc.tile_pool(name="psp", bufs=2, space="PSUM") as ppool,
        ):
            # --- resident weights ---------------------------------------
            wh_sb = cpool.tile([P, KH, H], BF16, tag="wh")
            nc.sync.dma_start(wh_sb[:], wh_d.rearrange("(kb p) c -> p kb c", p=P))
            wx_sb = cpool.tile([P, KF, H], BF16, tag="wx")
            nc.sync.dma_start(wx_sb[:], wx_d.rearrange("(kb p) c -> p kb c", p=P))
            wo_sb = cpool.tile([P, MH, O], BF16, tag="wo")
            nc.sync.dma_start(wo_sb[:], wo_d.rearrange("(kb p) c -> p kb c", p=P))
            b_sb = cpool.tile([P, KH], F32, tag="b")
            nc.sync.dma_start(b_sb[:], b_d.rearrange("(m p) -> p m", p=P))
            bo_sb = cpool.tile([P, OBK], F32, tag="bo")
            nc.sync.dma_start(bo_sb[:], bo_d.rearrange("(m p) -> p m", p=P))

            def emit_whole_kernel():
                stage_prev = None
                stage_cur = None
                pending_proj = []  # (window_idx, stage_tiles)

                def emit_proj_block(w_idx, stiles, ob):
                    pp = ppool.tile([P, pw], F32, tag="pproj", name="pproj")
                    if parity:
                        half = pw // 2
                        for par in range(2):
                            for m in range(MH):
                                nc.tensor.matmul(
                                    pp[:, par * half : (par + 1) * half],
                                    wo_sb[:, m, ob * P : (ob + 1) * P],
                                    stiles[m][par][:, :, 0:MB],
                                    start=(m == 0),
                                    stop=(m == MH - 1),
                                )
                    else:
                        for m in range(MH):
                            nc.tensor.matmul(
                                pp,
                                wo_sb[:, m, ob * P : (ob + 1) * P],
                                stiles[m][:, :, 0:MB],
                                start=(m == 0),
                                stop=(m == MH - 1),
                            )
                    osb = opool.tile([P, pw], F32, tag="osb", name="osb")
                    nc.vector.tensor_scalar_add(osb, pp, bo_sb[:, ob : ob + 1])
                    nc.sync.dma_start(out_d[w_idx, ob], osb)

                def state_slice(tiles_m, t_local):
                    if parity:
                        return tiles_m[t_local % 2][:, t_local // 2, :]
                    return tiles_m[:, t_local, :]

                for t in range(t_steps):
                    tl = t % w_steps
                    if tl == 0:
                        stage_prev = stage_cur
                        if parity:
                            stage_cur = [
                                [
                                    spool.tile(
                                        [P, w_steps // 2, B], BF16,
                                        tag=f"stage{m}p{par}", name=f"stage{m}p{par}",
                                    )
                                    for par in range(2)
                                ]
                                for m in range(MH)
                            ]
                        else:
                            stage_cur = [
                                spool.tile(
                                    [P, w_steps, B], BF16,
                                    tag=f"stage{m}", name=f"stage{m}",
                                )
                                for m in range(MH)
                            ]

                    if g_dma == 1:
                        xt_sb = xpool.tile([P, KF, B], BF16, tag="xt", name="xt")
                        nc.sync.dma_start(
                            xt_sb[:], xt_d[t].rearrange("(kb p) b -> p kb b", p=P)
                        )
                        xt_step = xt_sb
                    else:
                        if t % g_dma == 0:
                            xt_grp = xpool.tile(
                                [P, KF, g_dma, B], BF16, tag="xt", name="xt"
                            )
                            for kb in range(KF):
                                nc.sync.dma_start(
                                    xt_grp[:, kb],
                                    xt_d[
                                        bass.ds(t, g_dma),
                                        kb * P : (kb + 1) * P,
                                        :,
                                    ].rearrange("t p b -> p t b"),
                                )
                        xt_step = xt_grp[:, :, t % g_dma, :]

                    deferred = None  # (ps, prev, ptl) for group 0's k=7
                    for m in range(MH):
                        ps = zpool.tile([P, B], F32, tag="psz", name="psz")
                        do_defer = defer_k7 and m == 0 and t > 0
                        nlast = KF - 1 if t == 0 else KF + KH - 1
                        idx = 0
                        for kf in range(KF):
                            nc.tensor.matmul(
                                ps,
                                wx_sb[:, kf, m * P : (m + 1) * P],
                                xt_step[:, kf, :] if g_dma == 1 else xt_step[:, kf, :],
                                start=(idx == 0),
                                stop=(not do_defer and idx == nlast),
                            )
                            idx += 1
                        if t > 0:
                            prev = stage_cur if tl > 0 else stage_prev
                            ptl = (t - 1) % w_steps
                            ks = range(KH - 1) if do_defer else range(KH)
                            for k in ks:
                                nc.tensor.matmul(
                                    ps,
                                    wh_sb[:, k, m * P : (m + 1) * P],
                                    state_slice(prev[k], ptl),
                                    start=False,
                                    stop=(not do_defer and idx == nlast),
                                )
                                idx += 1
                        if do_defer:
                            deferred = (ps, prev, ptl)
                            continue
                        nc.scalar.activation(
                            state_slice(stage_cur[m], tl),
                            ps,
                            mybir.ActivationFunctionType.Tanh,
                            bias=b_sb[:, m : m + 1],
                        )
                        if deferred is not None and m == 1:
                            ps0, prev0, ptl0 = deferred
                            deferred = None
                            nc.tensor.matmul(
                                ps0,
                                wh_sb[:, KH - 1, 0:P],
                                state_slice(prev0[KH - 1], ptl0),
                                start=False,
                                stop=True,
                            )
                            nc.scalar.activation(
                                state_slice(stage_cur[0], tl),
                                ps0,
                                mybir.ActivationFunctionType.Tanh,
                                bias=b_sb[:, 0:1],
                            )

                    if (
                        pending_proj
                        and tl % proj_every == proj_every - 1
                        and tl // proj_every < OBK
                    ):
                        emit_proj_block(
                            pending_proj[0][0], pending_proj[0][1], tl // proj_every
                        )
                        if tl // proj_every == OBK - 1:
                            pending_proj.pop(0)

                    if tl == w_steps - 1:
                        pending_proj.append((t // w_steps, stage_cur))

                for w_idx, stiles in pending_proj:
                    for ob in range(OBK):
                        emit_proj_block(w_idx, stiles, ob)

            if reps > 1:
                with tc.For_i(0, reps, 1):
                    emit_whole_kernel()
            else:
                emit_whole_kernel()

    nc.compile()
    return nc


def build_program2(
    t_steps: int = T,
    w_steps: int = 64,
    zbufs: int = 1,
    proj_every: int = 2,
    reps: int = 1,
    defer_k7: bool = True,
    sbufs: int = 2,
    act_split: tuple = (1, 5, 1, 1),
    zgroup_bufs: tuple | None = None,
    g_dma: int = 1,
) -> bass.Bass:
    """Batched-activation variant.

    The baseline is ScalarE-bound: 8 per-step ACTIVATEs of [128,64] cost
    8*(64+352)/1.2 = 2.77us/step (the 352-cycle fixed overhead dominates at
    N=64), which matches the measured 2.86ms almost exactly.  Here the
    per-step z accumulators for several m-blocks share one PSUM bank
    ([128, g*64] f32 <= 2KB) so one ACTIVATE covers g m-blocks.  act_split
    partitions the 8 m-blocks into consecutive ACT groups; late groups stay
    small so the tanh feeding the next step's high-k matmuls is short.
    Requires b == 0 (no per-m bias possible in a batched ACT) -- the caller
    falls back to build_program otherwise.  Stage tiles merge all m-blocks
    ([P, w/2, KH, B] per parity) so one ACT write covers a group.
    """
    assert t_steps % w_steps == 0
    assert sum(act_split) == MH
    # start=True clears has_written for the WHOLE bank, so the deferred k7
    # (start=False, emitted after m1's group) only accumulates correctly if
    # m0 has its own bank -- i.e. act group 0 must be the singleton {m0}.
    assert not defer_k7 or act_split[0] == 1, "defer_k7 needs act_split[0]==1"
    if zgroup_bufs is None:
        # singleton groups' ACTs complete with big slack before their bank is
        # rewritten next step; only big groups (ACT near the step boundary)
        # need double-buffering against the next step's PE writes.
        zgroup_bufs = tuple(zbufs if g == 1 else max(zbufs, 2) for g in act_split)
    assert len(zgroup_bufs) == len(act_split)
    nw = t_steps // w_steps
    wh_half = w_steps // 2
    pw = w_steps * MB  # projection moving size per window
    half = pw // 2
    # m-group boundaries
    bounds = []
    lo = 0
    for g in act_split:
        bounds.append((lo, lo + g))
        lo += g
    grp_of = {}
    for gi, (mlo, mhi) in enumerate(bounds):
        for m in range(mlo, mhi):
            grp_of[m] = gi

    nc = bacc.Bacc()

    xt_d = nc.declare_dram_parameter("xt", [t_steps, F, B], BF16, isOutput=False)
    wh_d = nc.declare_dram_parameter("wh", [H, H], BF16, isOutput=False)
    wx_d = nc.declare_dram_parameter("wx", [F, H], BF16, isOutput=False)
    wo_d = nc.declare_dram_parameter("wout", [H, O], BF16, isOutput=False)
    bo_d = nc.declare_dram_parameter("boutvec", [O], F32, isOutput=False)
    out_d = nc.declare_dram_parameter("out", [nw, OBK, P, pw], F32, isOutput=True)

    with tile.TileContext(nc) as tc:
        with (
            tc.tile_pool(name="const", bufs=1) as cpool,
            tc.tile_pool(name="stage", bufs=sbufs) as spool,
            tc.tile_pool(name="xin", bufs=6) as xpool,
            tc.tile_pool(name="outsb", bufs=4) as opool,
            tc.tile_pool(name="psz1", bufs=1, space="PSUM") as zpool1,
            tc.tile_pool(name="psz2", bufs=2, space="PSUM") as zpool2,
            tc.tile_pool(name="psz3", bufs=3, space="PSUM") as zpool3,
            tc.tile_pool(name="psp", bufs=2, space="PSUM") as ppool,
        ):
            zpools = {1: zpool1, 2: zpool2, 3: zpool3}
            wh_sb = cpool.tile([P, KH, H], BF16, tag="wh")
            nc.sync.dma_start(wh_sb[:], wh_d.rearrange("(kb p) c -> p kb c", p=P))
            wx_sb = cpool.tile([P, KF, H], BF16, tag="wx")
            nc.sync.dma_start(wx_sb[:], wx_d.rearrange("(kb p) c -> p kb c", p=P))
            wo_sb = cpool.tile([P, MH, O], BF16, tag="wo")
            nc.sync.dma_start(wo_sb[:], wo_d.rearrange("(kb p) c -> p kb c", p=P))
            bo_sb = cpool.tile([P, OBK], F32, tag="bo")
            nc.sync.dma_start(bo_sb[:], bo_d.rearrange("(m p) -> p m", p=P))

            def emit_whole_kernel():
                stage_prev = None
                stage_cur = None
                pending_proj = []

                def emit_proj_block(w_idx, stiles, ob):
                    pp = ppool.tile([P, pw], F32, tag="pproj", name="pproj")
                    for par in range(2):
                        for m in range(MH):
                            nc.tensor.matmul(
                                pp[:, par * half : (par + 1) * half],
                                wo_sb[:, m, ob * P : (ob + 1) * P],
                                stiles[par][:, :, m, 0:MB],
                                start=(m == 0),
                                stop=(m == MH - 1),
                            )
                    osb = opool.tile([P, pw], F32, tag="osb", name="osb")
                    nc.vector.tensor_scalar_add(osb, pp, bo_sb[:, ob : ob + 1])
                    nc.sync.dma_start(out_d[w_idx, ob], osb)

                for t in range(t_steps):
                    tl = t % w_steps
                    par, row = tl % 2, tl // 2
                    if tl == 0:
                        stage_prev = stage_cur
                        stage_cur = [
                            spool.tile(
                                [P, wh_half, KH, B], BF16,
                                tag=f"stage{pp_}", name=f"stage{pp_}",
                            )
                            for pp_ in range(2)
                        ]

                    if g_dma == 1:
                        xt_sb = xpool.tile([P, KF, B], BF16, tag="xt", name="xt")
                        nc.sync.dma_start(
                            xt_sb[:], xt_d[t].rearrange("(kb p) b -> p kb b", p=P)
                        )
                        xt_step = xt_sb
                    else:
                        if t % g_dma == 0:
                            xt_grp = xpool.tile(
                                [P, KF, g_dma, B], BF16, tag="xt", name="xt"
                            )
                            for kb in range(KF):
                                nc.sync.dma_start(
                                    xt_grp[:, kb],
                                    xt_d[
                                        bass.ds(t, g_dma),
                                        kb * P : (kb + 1) * P,
                                        :,
                                    ].rearrange("t p b -> p t b"),
                                )
                        xt_step = xt_grp[:, :, t % g_dma, :]

                    if t > 0:
                        if tl > 0:
                            ppar, prow = (tl - 1) % 2, (tl - 1) // 2
                            prev_t = stage_cur
                        else:
                            ppar, prow = (w_steps - 1) % 2, (w_steps - 1) // 2
                            prev_t = stage_prev

                    ztiles = [
                        zpools[zgroup_bufs[gi]].tile(
                            [P, (mhi - mlo) * B], F32, tag=f"z{gi}",
                            name=f"z{gi}",
                        )
                        for gi, (mlo, mhi) in enumerate(bounds)
                    ]

                    deferred = None  # (zslice,) for m=0's k=KH-1
                    for m in range(MH):
                        gi = grp_of[m]
                        mlo, mhi = bounds[gi]
                        zs = ztiles[gi][:, (m - mlo) * B : (m - mlo + 1) * B]
                        do_defer = defer_k7 and m == 0 and t > 0
                        nlast = KF - 1 if t == 0 else KF + KH - 1
                        idx = 0
                        for kf in range(KF):
                            nc.tensor.matmul(
                                zs,
                                wx_sb[:, kf, m * P : (m + 1) * P],
                                xt_step[:, kf, :],
                                start=(idx == 0),
                                stop=(not do_defer and idx == nlast),
                            )
                            idx += 1
                        if t > 0:
                            ks = range(KH - 1) if do_defer else range(KH)
                            for k in ks:
                                nc.tensor.matmul(
                                    zs,
                                    wh_sb[:, k, m * P : (m + 1) * P],
                                    prev_t[ppar][:, prow, k, :],
                                    start=False,
                                    stop=(not do_defer and idx == nlast),
                                )
                                idx += 1
                        if do_defer:
                            deferred = zs
                        if deferred is not None and m == 1:
                            nc.tensor.matmul(
                                deferred,
                                wh_sb[:, KH - 1, 0:P],
                                prev_t[ppar][:, prow, KH - 1, :],
                                start=False,
                                stop=True,
                            )
                            deferred = None
                        # emit the ACT for any group whose last m-block is done;
                        # a singleton group 0 with a deferred k7 completes at m=1
                        for gj, (glo, ghi) in enumerate(bounds):
                            ready_at = ghi - 1
                            if defer_k7 and t > 0 and gj == 0:
                                ready_at = max(ready_at, 1)
                            if ready_at == m:
                                nc.scalar.activation(
                                    stage_cur[par][:, row, glo:ghi, :],
                                    ztiles[gj][:, 0 : (ghi - glo) * B],
                                    mybir.ActivationFunctionType.Tanh,
                                )

                    if (
                        pending_proj
                        and tl % proj_every == proj_every - 1
                        and tl // proj_every < OBK
                    ):
                        emit_proj_block(
                            pending_proj[0][0], pending_proj[0][1], tl // proj_every
                        )
                        if tl // proj_every == OBK - 1:
                            pending_proj.pop(0)

                    if tl == w_steps - 1:
                        pending_proj.append((t // w_steps, stage_cur))

                for w_idx, stiles in pending_proj:
                    for ob in range(OBK):
                        emit_proj_block(w_idx, stiles, ob)

            if reps > 1:
                with tc.For_i(0, reps, 1):
                    emit_whole_kernel()
            else:
                emit_whole_kernel()

    nc.compile()
    return nc


def _host_prep(x, Wx, Wh, b, Wout, bout, t_steps):
    """Build the 8 per-core input maps."""
    xt = np.ascontiguousarray(x[:, :, :t_steps].transpose(2, 1, 0)).astype(np_bf16)
    wh = Wh.astype(np_bf16)
    wx = Wx.astype(np_bf16)
    wo = Wout.astype(np_bf16)
    bv = np.ascontiguousarray(b, dtype=np.float32)
    bo = np.ascontiguousarray(bout, dtype=np.float32)
    in_maps = []
    for c in range(NCORES):
        xt_c = np.ascontiguousarray(np.roll(xt, -MB * c, axis=2))
        in_maps.append(
            {
                "xt": xt_c,
                "wh": wh,
                "wx": wx,
                "wout": wo,
                "bvec": bv,
                "boutvec": bo,
            }
        )
    return in_maps


def _assemble(results, t_steps, w_steps, parity=False):
    nw = t_steps // w_steps
    out = np.empty((B, t_steps, O), np.float32)
    for c in range(NCORES):
        if parity:
            arr = results[c]["out"].reshape(nw, OBK, P, 2, w_steps // 2, MB)
            # out[MB*c+j, w*W + tt*2 + par, ob*P + p] = arr[w, ob, p, par, tt, j]
            out[MB * c : MB * (c + 1)] = (
                arr.transpose(5, 0, 4, 3, 1, 2).reshape(MB, t_steps, O)
            )
        else:
            arr = results[c]["out"].reshape(nw, OBK, P, w_steps, MB)
            out[MB * c : MB * (c + 1)] = (
                arr.transpose(4, 0, 3, 1, 2).reshape(MB, t_steps, O)
            )
    return out


def build_final(reps: int = 1, v2: bool = True) -> bass.Bass:
    """The shipped program (v2 defaults); old path for nonzero b."""
    if v2:
        return build_program2(T, 64, reps=reps, zgroup_bufs=(1, 2, 2, 1))
    return build_program(T, 64, zbufs=6, parity=True, reps=reps)


def run(
    x, Wx, Wh, b, Wout, bout,
    t_steps=T, w_steps=64, zbufs=4, parity=True, trace=False, v2=None,
    v2_kwargs=None,
):
    if v2 is None:
        v2 = not np.any(np.asarray(b))
    if v2:
        kw = {"zgroup_bufs": (1, 2, 2, 1)}
        kw.update(v2_kwargs or {})
        nc = build_program2(t_steps, w_steps, **kw)
    else:
        nc = build_program(t_steps, w_steps, zbufs=zbufs, parity=parity)
    in_maps = _host_prep(x, Wx, Wh, b, Wout, bout, t_steps)
    if v2:
        for m in in_maps:
            m.pop("bvec", None)
    res = run_bass_kernel_spmd(nc, in_maps, list(range(NCORES)), trace=trace)
    out = _assemble(res.results, t_steps, w_steps, parity=True)
    return out, res


def kernel(x, Wx, Wh, b, Wout, bout):
    out, _ = run(
        np.asarray(x, dtype=np.float32),
        np.asarray(Wx, dtype=np.float32),
        np.asarray(Wh, dtype=np.float32),
        np.asarray(b, dtype=np.float32),
        np.asarray(Wout, dtype=np.float32),
        np.asarray(bout, dtype=np.float32),
    )
    return out



# revision 21
# speedup vs baseline: 1.6479x; 1.0060x over previous
"""Trainium2 Bass kernel for a basic tanh RNN + output projection.

Reference computation (all fp32):
    s_t = tanh(x[:, :, t] @ Wx + s_{t-1} @ Wh + b)      t = 0..T-1, s_{-1} = 0
    out[:, t, :] = s_t @ Wout + bout

Shapes: x (64, 256, 1024), Wx (256, 1024), Wh (1024, 1024), b (1024,),
        Wout (1024, 512), bout (512,)  ->  out (64, 1024, 512)

Strategy (8 NeuronCores):
  The T=1024 recurrence is sequential; per step the PE must reload all 80
  [128,128] stationary tiles (64 Wh + 16 Wx), which costs the same whether
  a core carries 8 or 64 batch rows, and per-step cross-core exchange is
  impossible (collectives have a ~20us latency floor).  So every core runs
  the full-batch recurrence (replicated; state kept transposed [H, B] on
  partitions so no per-step transposes are needed), and only the parallel
  work -- the output projection and the output writes -- is sharded by
  batch.  Each core receives x with the batch axis rotated so that its own
  8 batch columns sit at positions 0..7; all cores run one identical
  program (SPMD).

  Per step (bf16 matmul inputs, fp32 PSUM): for each of 8 hidden m-blocks,
  2 Wx + 8 Wh matmuls of [128,128]x[128,64] accumulate z.T in PSUM.  The
  previous version then ran one ScalarE tanh per m-block; at [128,64] an
  ACTIVATE costs (64+352)/1.2 ns -- the 352-cycle fixed overhead dominates
  -- so 8 per step saturate ScalarE at 2.77us/step, which matched the
  measured 2.86 ms almost exactly (ScalarE-bound, not PE-bound).  Now the
  z accumulators of several m-blocks share a PSUM bank (act groups
  m0 | m1..m4 | m5 | m6 | m7) and ONE ACTIVATE covers each group, cutting
  ScalarE to ~1.9us/step with the wide ACT retiring by ~80% of the step.
  The trailing groups stay singletons so each tanh feeding the next step's
  high-k matmuls is short (347 ns), and m0's k=7 matmul is deferred until
  after m1's group, giving tanh(m7) time to land (m0 must then own a bank
  by itself: start=True clears has_written for the whole bank, which would
  corrupt a deferred accumulate into a shared bank).  The wide group's ACT
  runs nearest the step boundary, so its bank is double-buffered against
  the next step's PE writes; singleton banks are reused every step (their
  ACTs retire early).  Stage state is
  one [128, w/2, 8, 64] bf16 tile per parity (even/odd steps alternate
  tiles, avoiding false WAR hazards); an ACT group writes [128, g*64]
  contiguously and the projection reads [w/2, 8] strided APs per m.  The
  Wh k-loop runs ascending so early weight loads have already-satisfied
  dependencies.  Every 64 steps the projection for the core's own 8 batch
  columns is folded in; its bias-add runs on VectorE -- keeping ScalarE
  exclusively on Tanh avoids ~2.7us activation-table reloads.  Batched
  tanh cannot apply a per-m bias ([P,1] only), so this path requires
  b == 0 (true per the spec, fill=zeros); kernel() falls back to the
  per-m-tanh program for nonzero b.  Local slope-measured 2.856 ms clean
  (sub-slopes within 0.6%; the old default baseline measures 2.86-4.7 ms
  under the same device-phase drift, 3.41 ms in a matched phase); on
  ScalarE-bound hardware the model predicts ~2.3-2.5 ms.  rel err 6.05e-3
  (bit-identical math to the previous version).
"""

import numpy as np
import ml_dtypes

import concourse.bass as bass
from concourse import bacc
import concourse.mybir as mybir
import concourse.tile as tile
from concourse.bass_utils import run_bass_kernel_spmd

B, F, T = 64, 256, 1024
H, O = 1024, 512
NCORES = 8
MB = B // NCORES  # own-batch columns per core (projection shard)
P = 128
KH, KF, MH, OBK = H // P, F // P, H // P, O // P  # 8, 2, 8, 4

BF16 = mybir.dt.bfloat16
F32 = mybir.dt.float32
np_bf16 = ml_dtypes.bfloat16


def build_program(
    t_steps: int = T,
    w_steps: int = 32,
    zbufs: int = 4,
    proj_every: int = 2,
    reps: int = 1,
    parity: bool = False,
    defer_k7: bool = False,
    sbufs: int = 2,
    g_dma: int = 1,
) -> bass.Bass:
    assert t_steps % w_steps == 0
    nw = t_steps // w_steps
    pw = w_steps * MB  # projection moving size per window

    nc = bacc.Bacc()

    xt_d = nc.declare_dram_parameter("xt", [t_steps, F, B], BF16, isOutput=False)
    wh_d = nc.declare_dram_parameter("wh", [H, H], BF16, isOutput=False)
    wx_d = nc.declare_dram_parameter("wx", [F, H], BF16, isOutput=False)
    wo_d = nc.declare_dram_parameter("wout", [H, O], BF16, isOutput=False)
    b_d = nc.declare_dram_parameter("bvec", [H], F32, isOutput=False)
    bo_d = nc.declare_dram_parameter("boutvec", [O], F32, isOutput=False)
    out_d = nc.declare_dram_parameter("out", [nw, OBK, P, pw], F32, isOutput=True)

    with tile.TileContext(nc) as tc:
        with (
            tc.tile_pool(name="const", bufs=1) as cpool,
            tc.tile_pool(name="stage", bufs=sbufs) as spool,
            tc.tile_pool(name="xin", bufs=max(2, 6 // g_dma)) as xpool,
            tc.tile_pool(name="outsb", bufs=4) as opool,
            tc.tile_pool(name="psz", bufs=zbufs, space="PSUM") as zpool,
            tc.tile_pool(name="psp", bufs=2, space="PSUM") as ppool,
        ):
            # --- resident weights ---------------------------------------
            wh_sb = cpool.tile([P, KH, H], BF16, tag="wh")
            nc.sync.dma_start(wh_sb[:], wh_d.rearrange("(kb p) c -> p kb c", p=P))
            wx_sb = cpool.tile([P, KF, H], BF16, tag="wx")
            nc.sync.dma_start(wx_sb[:], wx_d.rearrange("(kb p) c -> p kb c", p=P))
            wo_sb = cpool.tile([P, MH, O], BF16, tag="wo")
            nc.sync.dma_start(wo_sb[:], wo_d.rearrange("(kb p) c -> p kb c", p=P))
            b_sb = cpool.tile([P, KH], F32, tag="b")
            nc.sync.dma_start(b_sb[:], b_d.rearrange("(m p) -> p m", p=P))
            bo_sb = cpool.tile([P, OBK], F32, tag="bo")
            nc.sync.dma_start(bo_sb[:], bo_d.rearrange("(m p) -> p m", p=P))

            def emit_whole_kernel():
                stage_prev = None
                stage_cur = None
                pending_proj = []  # (window_idx, stage_tiles)

                def emit_proj_block(w_idx, stiles, ob):
                    pp = ppool.tile([P, pw], F32, tag="pproj", name="pproj")
                    if parity:
                        half = pw // 2
                        for par in range(2):
                            for m in range(MH):
                                nc.tensor.matmul(
                                    pp[:, par * half : (par + 1) * half],
                                    wo_sb[:, m, ob * P : (ob + 1) * P],
                                    stiles[m][par][:, :, 0:MB],
                                    start=(m == 0),
                                    stop=(m == MH - 1),
                                )
                    else:
                        for m in range(MH):
                            nc.tensor.matmul(
                                pp,
                                wo_sb[:, m, ob * P : (ob + 1) * P],
                                stiles[m][:, :, 0:MB],
                                start=(m == 0),
                                stop=(m == MH - 1),
                            )
                    osb = opool.tile([P, pw], F32, tag="osb", name="osb")
                    nc.vector.tensor_scalar_add(osb, pp, bo_sb[:, ob : ob + 1])
                    nc.sync.dma_start(out_d[w_idx, ob], osb)

                def state_slice(tiles_m, t_local):
                    if parity:
                        return tiles_m[t_local % 2][:, t_local // 2, :]
                    return tiles_m[:, t_local, :]

                for t in range(t_steps):
                    tl = t % w_steps
                    if tl == 0:
                        stage_prev = stage_cur
                        if parity:
                            stage_cur = [
                                [
                                    spool.tile(
                                        [P, w_steps // 2, B], BF16,
                                        tag=f"stage{m}p{par}", name=f"stage{m}p{par}",
                                    )
                                    for par in range(2)
                                ]
                                for m in range(MH)
                            ]
                        else:
                            stage_cur = [
                                spool.tile(
                                    [P, w_steps, B], BF16,
                                    tag=f"stage{m}", name=f"stage{m}",
                                )
                                for m in range(MH)
                            ]

                    if g_dma == 1:
                        xt_sb = xpool.tile([P, KF, B], BF16, tag="xt", name="xt")
                        nc.sync.dma_start(
                            xt_sb[:], xt_d[t].rearrange("(kb p) b -> p kb b", p=P)
                        )
                        xt_step = xt_sb
                    else:
                        if t % g_dma == 0:
                            xt_grp = xpool.tile(
                                [P, KF, g_dma, B], BF16, tag="xt", name="xt"
                            )
                            for kb in range(KF):
                                nc.sync.dma_start(
                                    xt_grp[:, kb],
                                    xt_d[
                                        bass.ds(t, g_dma),
                                        kb * P : (kb + 1) * P,
                                        :,
                                    ].rearrange("t p b -> p t b"),
                                )
                        xt_step = xt_grp[:, :, t % g_dma, :]

                    deferred = None  # (ps, prev, ptl) for group 0's k=7
                    for m in range(MH):
                        ps = zpool.tile([P, B], F32, tag="psz", name="psz")
                        do_defer = defer_k7 and m == 0 and t > 0
                        nlast = KF - 1 if t == 0 else KF + KH - 1
                        idx = 0
                        for kf in range(KF):
                            nc.tensor.matmul(
                                ps,
                                wx_sb[:, kf, m * P : (m + 1) * P],
                                xt_step[:, kf, :] if g_dma == 1 else xt_step[:, kf, :],
                                start=(idx == 0),
                                stop=(not do_defer and idx == nlast),
                            )
                            idx += 1
                        if t > 0:
                            prev = stage_cur if tl > 0 else stage_prev
                            ptl = (t - 1) % w_steps
                            ks = range(KH - 1) if do_defer else range(KH)
                            for k in ks:
                                nc.tensor.matmul(
                                    ps,
                                    wh_sb[:, k, m * P : (m + 1) * P],
                                    state_slice(prev[k], ptl),
                                    start=False,
                                    stop=(not do_defer and idx == nlast),
                                )
                                idx += 1
                        if do_defer:
                            deferred = (ps, prev, ptl)
                            continue
                        nc.scalar.activation(
                            state_slice(stage_cur[m], tl),
                            ps,
                            mybir.ActivationFunctionType.Tanh,
                            bias=b_sb[:, m : m + 1],
                        )
                        if deferred is not None and m == 1:
                            ps0, prev0, ptl0 = deferred
                            deferred = None
                            nc.tensor.matmul(
                                ps0,
                                wh_sb[:, KH - 1, 0:P],
                                state_slice(prev0[KH - 1], ptl0),
                                start=False,
                                stop=True,
                            )
                            nc.scalar.activation(
                                state_slice(stage_cur[0], tl),
                                ps0,
                                mybir.ActivationFunctionType.Tanh,
                                bias=b_sb[:, 0:1],
                            )

                    if (
                        pending_proj
                        and tl % proj_every == proj_every - 1
                        and tl // proj_every < OBK
                    ):
                        emit_proj_block(
                            pending_proj[0][0], pending_proj[0][1], tl // proj_every
                        )
                        if tl // proj_every == OBK - 1:
                            pending_proj.pop(0)

                    if tl == w_steps - 1:
                        pending_proj.append((t // w_steps, stage_cur))

                for w_idx, stiles in pending_proj:
                    for ob in range(OBK):
                        emit_proj_block(w_idx, stiles, ob)

            if reps > 1:
                with tc.For_i(0, reps, 1):
                    emit_whole_kernel()
            else:
                emit_whole_kernel()

    nc.compile()
    return nc


def build_program2(
    t_steps: int = T,
    w_steps: int = 64,
    zbufs: int = 1,
    proj_every: int = 2,
    reps: int = 1,
    defer_k7: bool = True,
    sbufs: int = 2,
    act_split: tuple = (1, 5, 1, 1),
    zgroup_bufs: tuple | None = None,
    g_dma: int = 1,
) -> bass.Bass:
    """Batched-activation variant.

    The baseline is ScalarE-bound: 8 per-step ACTIVATEs of [128,64] cost
    8*(64+352)/1.2 = 2.77us/step (the 352-cycle fixed overhead dominates at
    N=64), which matches the measured 2.86ms almost exactly.  Here the
    per-step z accumulators for several m-blocks share one PSUM bank
    ([128, g*64] f32 <= 2KB) so one ACTIVATE covers g m-blocks.  act_split
    partitions the 8 m-blocks into consecutive ACT groups; late groups stay
    small so the tanh feeding the next step's high-k matmuls is short.
    Requires b == 0 (no per-m bias possible in a batched ACT) -- the caller
    falls back to build_program otherwise.  Stage tiles merge all m-blocks
    ([P, w/2, KH, B] per parity) so one ACT write covers a group.
    """
    assert t_steps % w_steps == 0
    assert sum(act_split) == MH
    # start=True clears has_written for the WHOLE bank, so the deferred k7
    # (start=False, emitted after m1's group) only accumulates correctly if
    # m0 has its own bank -- i.e. act group 0 must be the singleton {m0}.
    assert not defer_k7 or act_split[0] == 1, "defer_k7 needs act_split[0]==1"
    if zgroup_bufs is None:
        # singleton groups' ACTs complete with big slack before their bank is
        # rewritten next step; only big groups (ACT near the step boundary)
        # need double-buffering against the next step's PE writes.
        zgroup_bufs = tuple(zbufs if g == 1 else max(zbufs, 2) for g in act_split)
    assert len(zgroup_bufs) == len(act_split)
    nw = t_steps // w_steps
    wh_half = w_steps // 2
    pw = w_steps * MB  # projection moving size per window
    half = pw // 2
    # m-group boundaries
    bounds = []
    lo = 0
    for g in act_split:
        bounds.append((lo, lo + g))
        lo += g
    grp_of = {}
    for gi, (mlo, mhi) in enumerate(bounds):
        for m in range(mlo, mhi):
            grp_of[m] = gi

    nc = bacc.Bacc()

    xt_d = nc.declare_dram_parameter("xt", [t_steps, F, B], BF16, isOutput=False)
    wh_d = nc.declare_dram_parameter("wh", [H, H], BF16, isOutput=False)
    wx_d = nc.declare_dram_parameter("wx", [F, H], BF16, isOutput=False)
    wo_d = nc.declare_dram_parameter("wout", [H, O], BF16, isOutput=False)
    bo_d = nc.declare_dram_parameter("boutvec", [O], F32, isOutput=False)
    out_d = nc.declare_dram_parameter("out", [nw, OBK, P, pw], F32, isOutput=True)

    with tile.TileContext(nc) as tc:
        with (
            tc.tile_pool(name="const", bufs=1) as cpool,
            tc.tile_pool(name="stage", bufs=sbufs) as spool,
            tc.tile_pool(name="xin", bufs=6) as xpool,
            tc.tile_pool(name="outsb", bufs=4) as opool,
            tc.tile_pool(name="psz1", bufs=1, space="PSUM") as zpool1,
            tc.tile_pool(name="psz2", bufs=2, space="PSUM") as zpool2,
            tc.tile_pool(name="psz3", bufs=3, space="PSUM") as zpool3,
            tc.tile_pool(name="psp", bufs=2, space="PSUM") as ppool,
        ):
            zpools = {1: zpool1, 2: zpool2, 3: zpool3}
            wh_sb = cpool.tile([P, KH, H], BF16, tag="wh")
            nc.sync.dma_start(wh_sb[:], wh_d.rearrange("(kb p) c -> p kb c", p=P))
            wx_sb = cpool.tile([P, KF, H], BF16, tag="wx")
            nc.sync.dma_start(wx_sb[:], wx_d.rearrange("(kb p) c -> p kb c", p=P))
            wo_sb = cpool.tile([P, MH, O], BF16, tag="wo")
            nc.sync.dma_start(wo_sb[:], wo_d.rearrange("(kb p) c -> p kb c", p=P))
            bo_sb = cpool.tile([P, OBK], F32, tag="bo")
            nc.sync.dma_start(bo_sb[:], bo_d.rearrange("(m p) -> p m", p=P))

            def emit_whole_kernel():
                stage_prev = None
                stage_cur = None
                pending_proj = []

                def emit_proj_block(w_idx, stiles, ob):
                    pp = ppool.tile([P, pw], F32, tag="pproj", name="pproj")
                    for par in range(2):
                        for m in range(MH):
                            nc.tensor.matmul(
                                pp[:, par * half : (par + 1) * half],
                                wo_sb[:, m, ob * P : (ob + 1) * P],
                                stiles[par][:, :, m, 0:MB],
                                start=(m == 0),
                                stop=(m == MH - 1),
                            )
                    osb = opool.tile([P, pw], F32, tag="osb", name="osb")
                    nc.vector.tensor_scalar_add(osb, pp, bo_sb[:, ob : ob + 1])
                    nc.sync.dma_start(out_d[w_idx, ob], osb)

                for t in range(t_steps):
                    tl = t % w_steps
                    par, row = tl % 2, tl // 2
                    if tl == 0:
                        stage_prev = stage_cur
                        stage_cur = [
                            spool.tile(
                                [P, wh_half, KH, B], BF16,
                                tag=f"stage{pp_}", name=f"stage{pp_}",
                            )
                            for pp_ in range(2)
                        ]

                    if g_dma == 1:
                        xt_sb = xpool.tile([P, KF, B], BF16, tag="xt", name="xt")
                        nc.sync.dma_start(
                            xt_sb[:], xt_d[t].rearrange("(kb p) b -> p kb b", p=P)
                        )
                        xt_step = xt_sb
                    else:
                        if t % g_dma == 0:
                            xt_grp = xpool.tile(
                                [P, KF, g_dma, B], BF16, tag="xt", name="xt"
                            )
                            for kb in range(KF):
                                nc.sync.dma_start(
                                    xt_grp[:, kb],
                                    xt_d[
                                        bass.ds(t, g_dma),
                                        kb * P : (kb + 1) * P,
                                        :,
                                    ].rearrange("t p b -> p t b"),
                                )
                        xt_step = xt_grp[:, :, t % g_dma, :]

                    if t > 0:
                        if tl > 0:
                            ppar, prow = (tl - 1) % 2, (tl - 1) // 2
                            prev_t = stage_cur
                        else:
                            ppar, prow = (w_steps - 1) % 2, (w_steps - 1) // 2
                            prev_t = stage_prev

                    ztiles = [
                        zpools[zgroup_bufs[gi]].tile(
                            [P, (mhi - mlo) * B], F32, tag=f"z{gi}",
                            name=f"z{gi}",
                        )
                        for gi, (mlo, mhi) in enumerate(bounds)
                    ]

                    deferred = None  # (zslice,) for m=0's k=KH-1
                    for m in range(MH):
                        gi = grp_of[m]
                        mlo, mhi = bounds[gi]
                        zs = ztiles[gi][:, (m - mlo) * B : (m - mlo + 1) * B]
                        do_defer = defer_k7 and m == 0 and t > 0
                        nlast = KF - 1 if t == 0 else KF + KH - 1
                        idx = 0
                        for kf in range(KF):
                            nc.tensor.matmul(
                                zs,
                                wx_sb[:, kf, m * P : (m + 1) * P],
                                xt_step[:, kf, :],
                                start=(idx == 0),
                                stop=(not do_defer and idx == nlast),
                            )
                            idx += 1
                        if t > 0:
                            ks = range(KH - 1) if do_defer else range(KH)
                            for k in ks:
                                nc.tensor.matmul(
                                    zs,
                                    wh_sb[:, k, m * P : (m + 1) * P],
                                    prev_t[ppar][:, prow, k, :],
                                    start=False,
                                    stop=(not do_defer and idx == nlast),
                                )
                                idx += 1
                        if do_defer:
                            deferred = zs
                        if deferred is not None and m == 1:
                            nc.tensor.matmul(
                                deferred,
                                wh_sb[:, KH - 1, 0:P],
                                prev_t[ppar][:, prow, KH - 1, :],
                                start=False,
                                stop=True,
                            )
                            deferred = None
                        # emit the ACT for any group whose last m-block is done;
                        # a singleton group 0 with a deferred k7 completes at m=1
                        for gj, (glo, ghi) in enumerate(bounds):
                            ready_at = ghi - 1
                            if defer_k7 and t > 0 and gj == 0:
                                ready_at = max(ready_at, 1)
                            if ready_at == m:
                                nc.scalar.activation(
                                    stage_cur[par][:, row, glo:ghi, :],
                                    ztiles[gj][:, 0 : (ghi - glo) * B],
                                    mybir.ActivationFunctionType.Tanh,
                                )

                    if (
                        pending_proj
                        and tl % proj_every == proj_every - 1
                        and tl // proj_every < OBK
                    ):
                        emit_proj_block(
                            pending_proj[0][0], pending_proj[0][1], tl // proj_every
                        )
                        if tl // proj_every == OBK - 1:
                            pending_proj.pop(0)

                    if tl == w_steps - 1:
                        pending_proj.append((t // w_steps, stage_cur))

                for w_idx, stiles in pending_proj:
                    for ob in range(OBK):
                        emit_proj_block(w_idx, stiles, ob)

            if reps > 1:
                with tc.For_i(0, reps, 1):
                    emit_whole_kernel()
            else:
                emit_whole_kernel()

    nc.compile()
    return nc


def _host_prep(x, Wx, Wh, b, Wout, bout, t_steps):
    """Build the 8 per-core input maps."""
    xt = np.ascontiguousarray(x[:, :, :t_steps].transpose(2, 1, 0)).astype(np_bf16)
    wh = Wh.astype(np_bf16)
    wx = Wx.astype(np_bf16)
    wo = Wout.astype(np_bf16)
    bv = np.ascontiguousarray(b, dtype=np.float32)
    bo = np.ascontiguousarray(bout, dtype=np.float32)
    in_maps = []
    for c in range(NCORES):
        xt_c = np.ascontiguousarray(np.roll(xt, -MB * c, axis=2))
        in_maps.append(
            {
                "xt": xt_c,
                "wh": wh,
                "wx": wx,
                "wout": wo,
                "bvec": bv,
                "boutvec": bo,
            }
        )
    return in_maps


def _assemble(results, t_steps, w_steps, parity=False):
    nw = t_steps // w_steps
    out = np.empty((B, t_steps, O), np.float32)
    for c in range(NCORES):
        if parity:
            arr = results[c]["out"].reshape(nw, OBK, P, 2, w_steps // 2, MB)
            # out[MB*c+j, w*W + tt*2 + par, ob*P + p] = arr[w, ob, p, par, tt, j]
            out[MB * c : MB * (c + 1)] = (
                arr.transpose(5, 0, 4, 3, 1, 2).reshape(MB, t_steps, O)
            )
        else:
            arr = results[c]["out"].reshape(nw, OBK, P, w_steps, MB)
            out[MB * c : MB * (c + 1)] = (
                arr.transpose(4, 0, 3, 1, 2).reshape(MB, t_steps, O)
            )
    return out


def build_final(reps: int = 1, v2: bool = True) -> bass.Bass:
    """The shipped program (v2 defaults); old path for nonzero b."""
    if v2:
        return build_program2(T, 64, reps=reps, act_split=(1, 4, 1, 1, 1))
    return build_program(T, 64, zbufs=6, parity=True, reps=reps)


def run(
    x, Wx, Wh, b, Wout, bout,
    t_steps=T, w_steps=64, zbufs=4, parity=True, trace=False, v2=None,
    v2_kwargs=None,
):
    if v2 is None:
        v2 = not np.any(np.asarray(b))
    if v2:
        kw = {"act_split": (1, 4, 1, 1, 1)}
        kw.update(v2_kwargs or {})
        nc = build_program2(t_steps, w_steps, **kw)
    else:
        nc = build_program(t_steps, w_steps, zbufs=zbufs, parity=parity)
    in_maps = _host_prep(x, Wx, Wh, b, Wout, bout, t_steps)
    if v2:
        for m in in_maps:
            m.pop("bvec", None)
    res = run_bass_kernel_spmd(nc, in_maps, list(range(NCORES)), trace=trace)
    out = _assemble(res.results, t_steps, w_steps, parity=True)
    return out, res


def kernel(x, Wx, Wh, b, Wout, bout):
    out, _ = run(
        np.asarray(x, dtype=np.float32),
        np.asarray(Wx, dtype=np.float32),
        np.asarray(Wh, dtype=np.float32),
        np.asarray(b, dtype=np.float32),
        np.asarray(Wout, dtype=np.float32),
        np.asarray(bout, dtype=np.float32),
    )
    return out

